# revision 26
# baseline (speedup 1.0000x reference)
"""Haar wavelet (2x2 block) decomposition kernel for 8 Trainium2 NeuronCores.

Input  x: [16, 32, 512, 512] f32
Output  : [16, 128, 256, 256] f32 = concat([pooled, diffH, diffV, diffD], axis=1)

Sharding: pure data parallel over the batch axis — core i handles batches
[2i, 2i+1] (64 images of 512x512 per core).

Per-core dataflow (all fp32), `ipi` images per iteration, P = 128/ipi
partitions per image, R = 512/P input rows per partition:
  load X [128, R*512]   (one contiguous R*512*4-byte run per partition)
  s = E + O, d = E - O          (row butterfly, DVE)
  po = (s_e + s_o) * 0.25       (column butterfly, DVE + ACT scale)
  dv = (s_e - s_o) * 0.5
  dh = (d_e + d_o) * 0.5
  dd =  d_e - d_o
  one fused store of all 4 planes (4 contiguous runs per partition)
With inplace=True the output overwrites the X tile (X is dead after the
row butterfly), halving SBUF footprint so more buffers fit.

Tuning history (slope-protocol HW measurements, see bench.py/compare.py):
the kernel is HBM-bound.  Per-NC rates measured via DMA-only variants:
pure loads 345 GB/s, pure stores ~340 GB/s, but mixed 50/50 R+W traffic
only ~323 GB/s — and that mixed-traffic rate is the wall: a DMA-only
kernel (no compute) times identically to the full kernel, store
descriptor structure is irrelevant (a perfectly-linear store AP times
the same as the 4-runs-per-partition real one), and forcing coarse
unidirectional bursts by putting both directions on one HWDGE ring in
FIFO alternation does not help.  What did help: ipi 2 -> 4 (2 MiB -> 4
MiB DMAs) and deeper X buffering (bufs 3 -> 5, enabled by in-place
output reuse), worth ~7% combined in an interleaved A/B measurement
(444.7 -> 413.9 us/core).  ipi=8 measured no better than ipi=4 and fits
fewer buffers; deeper ipi=2 buffering did not help (427.8 us).

The walrus build available here only accepts ONE sync-wait per instruction
(setupSyncWait: "Too many sync wait commands"), while Tile freely attaches
several.  _split_multi_waits() post-processes the serialized BIR, hoisting
all-but-one wait of every instruction onto single-wait NoOps inserted just
before it (same engine, so per-engine program order is preserved).
"""

import functools

import numpy as np
import orjson

import concourse.bass as bass
import concourse.mybir as mybir
from concourse.tile import TileContext

_N_CORES = 8
_B, _C, _H, _W = 16, 32, 512, 512
_BPC = _B // _N_CORES  # batches per core
_IMGS = _BPC * _C  # images per core
_F32 = mybir.dt.float32

# default per-core pipeline config (see _build_nc)
_DEF = dict(ipi=4, inplace=True, bufs=5, sd_bufs=1, o_bufs=2, sd_bf16=False)


def _split_multi_waits(j: dict) -> dict:
    for fn in j["functions"]:
        for blk in fn["blocks"]:
            out = []
            for ins in blk["instructions"]:
                si = ins.get("sync_info")
                waits = (si or {}).get("on_wait") or []
                if len(waits) > 1:
                    for k, w in enumerate(waits[:-1]):
                        out.append(
                            {
                                "debug": ins.get("debug", 0),
                                "engine": ins["engine"],
                                "ins": [],
                                "outs": [],
                                "name": f"{ins['name']}__w{k}",
                                "opcode": "NoOp",
                                "text_hint": "split_wait",
                                "sync_info": {"on_update": [], "on_wait": [w]},
                            }
                        )
                    si["on_wait"] = [waits[-1]]
                out.append(ins)
            blk["instructions"] = out
    return j


if not getattr(bass.Bass.to_json_bytes, "_haar_split_patch", False):
    _orig_to_json_bytes = bass.Bass.to_json_bytes

    def _patched_to_json_bytes(self):
        j = orjson.loads(_orig_to_json_bytes(self))
        _split_multi_waits(j)
        return orjson.dumps(j)

    _patched_to_json_bytes._haar_split_patch = True
    bass.Bass.to_json_bytes = _patched_to_json_bytes


@functools.lru_cache(maxsize=None)
def _build_nc(
    reps=1, ipi=None, inplace=None, bufs=None, sd_bufs=None, o_bufs=None, sd_bf16=None,
    mode="full", chunks=None,
) -> bass.Bass:
    ipi = _DEF["ipi"] if ipi is None else ipi
    inplace = _DEF["inplace"] if inplace is None else inplace
    bufs = _DEF["bufs"] if bufs is None else bufs
    sd_bufs = _DEF["sd_bufs"] if sd_bufs is None else sd_bufs
    o_bufs = _DEF["o_bufs"] if o_bufs is None else o_bufs
    sd_bf16 = _DEF["sd_bf16"] if sd_bf16 is None else sd_bf16
    chunks = _DEF.get("chunks", 1) if chunks is None else chunks
    sd_dt = mybir.dt.bfloat16 if sd_bf16 else _F32

    P = 128 // ipi  # partitions per image
    R = _H // P  # input rows per partition
    A = R // 2  # output rows (row-pairs) per partition
    FW = A * _W  # free size of s/d per partition
    HP = FW // 2  # free size of one output plane per partition

    nc = bass.Bass()
    x = nc.dram_tensor("x", [_IMGS, _H, _W], _F32, kind="ExternalInput")
    y = nc.dram_tensor("y", [4 * _IMGS, _H // 2, _W // 2], _F32, kind="ExternalOutput")
    yv = y.rearrange("(b k c) h w -> b c k (h w)", b=_BPC, k=4)

    with TileContext(nc) as tc:
        with tc.tile_pool(name="sbuf", bufs=bufs) as pool:

            def body():
                # Loads go on the SP HWDGE ring, stores on the ACT ring so
                # both rings drive the SDMA pool concurrently.
                x_tiles = []
                for img0 in range(0, _IMGS, ipi):
                    if mode in ("stores", "storespure") and img0 >= bufs * ipi:
                        X = x_tiles[(img0 // ipi) % bufs]
                    else:
                        X = pool.tile([128, R * _W], _F32, tag="X")
                        x_tiles.append(X)
                        if mode == "storespure":
                            nc.vector.memset(X, 0.0)
                    if mode in ("full", "loads", "dma", "dmaser", "dmalin") or (
                        mode == "stores" and img0 < bufs * ipi
                    ):
                        nc.sync.dma_start(
                            out=X,
                            in_=x[img0 : img0 + ipi].rearrange(
                                "i (p a) w -> (i p) (a w)", p=P, a=R
                            ),
                        )
                    if mode in ("stores", "storespure", "dma", "dmaser"):
                        b, c0 = divmod(img0, _C)
                        eng = nc.sync if mode == "dmaser" else nc.scalar
                        eng.dma_start(
                            out=yv[b, c0 : c0 + ipi].rearrange(
                                "i k (p aw) -> (i p) k aw", p=P
                            ),
                            in_=X.rearrange("q (k aw) -> q k aw", k=4),
                        )
                    if mode == "dmalin":
                        # timing probe: same bytes, one contiguous run/partition
                        g = img0 // ipi
                        ylin = y.rearrange("(g a) h w -> g (a h) w", a=4 * ipi)[
                            g
                        ].rearrange("(p r) w -> p (r w)", p=128)
                        nc.scalar.dma_start(out=ylin, in_=X[:, : ylin.shape[1]])
                    if mode != "full":
                        continue
                    # all four results live in ONE tile so the outputs ship
                    # as a single fused store; with inplace=True that tile is
                    # X itself (dead after the row butterfly).
                    O = (
                        X
                        if inplace
                        else pool.tile([128, R * _W], _F32, tag="O", bufs=o_bufs)
                    )
                    # per partition q: R rows = (a = row-pair, eo = even/odd).
                    # Processed in `chunks` row-chunks so s/d can be small.
                    # O is laid out CHUNK-major ([t][k][cs]) so that with
                    # inplace=True chunk t only overwrites the X region it
                    # itself just consumed — later chunks' X rows stay
                    # intact.  The store AP regathers planes below.
                    ca = A // chunks  # row-pairs per chunk
                    cs = ca * _W // 2  # chunk's span within each plane
                    for t in range(chunks):
                        Xc = X[:, t * ca * 2 * _W : (t + 1) * ca * 2 * _W]
                        Xv = Xc.rearrange("q (a eo w) -> q eo a w", a=ca, eo=2)
                        # s/d lifetime is entirely within DVE program order
                        # (written by the row butterfly, read only by the
                        # column butterfly), so sd_bufs=1 causes no stalls.
                        s = pool.tile([128, ca * _W], sd_dt, tag="s", bufs=sd_bufs)
                        d = pool.tile([128, ca * _W], sd_dt, tag="d", bufs=sd_bufs)
                        nc.vector.tensor_add(out=s, in0=Xv[:, 0], in1=Xv[:, 1])
                        nc.vector.tensor_sub(out=d, in0=Xv[:, 0], in1=Xv[:, 1])
                        # column butterfly: split free into (x, v), v = even/odd col
                        sr = s.rearrange("q (x v) -> q v x", v=2)
                        dr = d.rearrange("q (x v) -> q v x", v=2)
                        po = O[:, (t * 4 + 0) * cs : (t * 4 + 1) * cs]
                        dh = O[:, (t * 4 + 1) * cs : (t * 4 + 2) * cs]
                        dv = O[:, (t * 4 + 2) * cs : (t * 4 + 3) * cs]
                        dd = O[:, (t * 4 + 3) * cs : (t * 4 + 4) * cs]
                        nc.vector.tensor_add(out=po, in0=sr[:, 0], in1=sr[:, 1])
                        nc.vector.tensor_add(out=dh, in0=dr[:, 0], in1=dr[:, 1])
                        nc.vector.tensor_sub(out=dv, in0=sr[:, 0], in1=sr[:, 1])
                        nc.vector.tensor_sub(out=dd, in0=dr[:, 0], in1=dr[:, 1])
                        nc.scalar.mul(po, po, 0.25)
                        nc.scalar.mul(dh, dh, 0.5)
                        nc.scalar.mul(dv, dv, 0.5)
                    b, c0 = divmod(img0, _C)
                    nc.scalar.dma_start(
                        out=yv[b, c0 : c0 + ipi].rearrange(
                            "i k (p t c) -> (i p) k t c", p=P, t=chunks
                        ),
                        in_=O.rearrange("q (t k c) -> q k t c", t=chunks, k=4),
                    )

            if reps == 1:
                body()
            else:
                # HW repeat loop for slope-based timing (hw_slope.py)
                with tc.For_i(0, reps):
                    body()
    return nc


@functools.lru_cache(maxsize=None)
def _build_runner(
    reps=1, ipi=None, inplace=None, bufs=None, sd_bufs=None, o_bufs=None, sd_bf16=None
):
    """Compile once; return a callable shards -> full output.

    Mirrors bass2jax.run_bass_via_pjrt's multi-core path (shard_map over the
    8 axon devices, donated zero output buffers), but keeps the jitted
    function alive so repeated kernel() calls don't recompile the NEFF.
    """
    import jax
    from jax.sharding import Mesh, PartitionSpec, NamedSharding
    from jax.experimental.shard_map import shard_map
    from concourse import bass2jax

    nc = _build_nc(reps, ipi, inplace, bufs, sd_bufs, o_bufs, sd_bf16)
    partition_name = nc.partition_id_tensor.name if nc.partition_id_tensor else None
    in_names, out_names, out_avals = [], [], []
    for alloc in nc.m.functions[0].allocations:
        if not isinstance(alloc, mybir.MemoryLocationSet):
            continue
        name = alloc.memorylocations[0].name
        if alloc.kind == "ExternalInput":
            if name != partition_name:
                in_names.append(name)
        elif alloc.kind == "ExternalOutput":
            out_names.append(name)
            out_avals.append(
                jax.core.ShapedArray(
                    tuple(alloc.tensor_shape), mybir.dt.np(alloc.dtype)
                )
            )
    n_params = len(in_names)
    n_outs = len(out_names)
    all_in_names = in_names + out_names + ([partition_name] if partition_name else [])

    def _body(*args):
        operands = list(args)
        if partition_name is not None:
            operands.append(bass2jax.partition_id_tensor())
        outs = bass2jax._bass_exec_p.bind(
            *operands,
            out_avals=tuple(out_avals),
            in_names=tuple(all_in_names),
            out_names=tuple(out_names),
            lowering_input_output_aliases=(),
            sim_require_finite=True,
            sim_require_nnan=True,
            nc=nc,
        )
        return tuple(outs)

    bass2jax.install_neuronx_cc_hook()
    devices = jax.devices()[:_N_CORES]
    assert len(devices) == _N_CORES, f"need {_N_CORES} devices, got {len(devices)}"
    mesh = Mesh(np.asarray(devices), ("core",))
    in_specs = (PartitionSpec("core"),) * (n_params + n_outs)
    out_specs = (PartitionSpec("core"),) * n_outs
    sharded = jax.jit(
        shard_map(
            _body, mesh=mesh, in_specs=in_specs, out_specs=out_specs, check_rep=False
        ),
        donate_argnums=tuple(range(n_params, n_params + n_outs)),
        keep_unused=True,
    )
    out_shape = out_avals[0].shape
    zero_shape = (_N_CORES * out_shape[0], *out_shape[1:])
    sh = NamedSharding(mesh, PartitionSpec("core"))
    # allocate + fill the donated output buffer on-device: avoids a 512 MiB
    # host->device transfer of zeros per call
    make_zeros = jax.jit(
        lambda: jax.numpy.zeros(zero_shape, np.float32), out_shardings=sh
    )

    def run(x_global: np.ndarray) -> np.ndarray:
        (out,) = sharded(x_global, make_zeros())
        return np.asarray(out)

    return run


def kernel(x) -> np.ndarray:
    x = np.ascontiguousarray(np.asarray(x), dtype=np.float32)
    assert x.shape == (_B, _C, _H, _W), x.shape
    x_global = x.reshape(_N_CORES * _IMGS, _H, _W)  # view, no copy
    out = _build_runner()(x_global)  # [8*4*_IMGS, 256, 256], core-major
    return out.reshape(_B, 4 * _C, _H // 2, _W // 2)
